# revision 2
# baseline (speedup 1.0000x reference)
"""Trainium2 Bass kernel for nn_EvMLP (segment_reduce EvNorm + invariant MLP).

Self-contained: hardcodes shapes/sharding. Accepts FULL inputs, returns FULL
output; shards the node dim N across 8 NeuronCores (pure data parallel).

Math (per row of ten [N, 592]):
  x10 = ten[:128]; eq = ten[128:]
  sumsq[c] = sum of eq^2 over segment c   (64x3, 32x5, 16x7 runs)
  d = sqrt(sumsq+1);  x11 = d-1;  x2 = eq / d[seg]
  x1 = [x10, x11]  (240)
  h = LN(x1@w1, g1, b1); h = silu(h@w2+b2); h = LN(h, g2, b2n); h = h@w3+b3
  out = [h, x2]

Key implementation ideas:
  - rows-on-partitions for the eq path (squares / segment reduce / divide)
  - feature-major MLP via PE transposes; LayerNorm mean-centering folded into
    the weights host-side (w @ C); LN variance via ones/128 matmul broadcast
  - rsqrt without ACT table switching: quake-style seed computed with ACT
    Identity dtype-conversion trick + 2 custom fused DVE Newton steps
  - only Silu/Square/Identity/Relu activation funcs -> single ACT table set
"""
import sys

sys.path.insert(0, "/opt/trn_rl_repo")

import numpy as np

import concourse.bass as bass
import concourse.bacc as bacc
import concourse.tile as tile
from concourse import mybir
from concourse.bass_utils import run_bass_kernel_spmd

f32 = mybir.dt.float32
i32 = mybir.dt.int32

# ---------------------------------------------------------------- constants --
N = 100000
DIM = 592
N_INV = 128
N_EQ_CH = 112
N_EQ = 464
EPS = 1e-5
N_CORES = 8
BLOCKS_PER_CORE = 98                      # 98*128 = 12544 rows/core
ROWS_PER_CORE = BLOCKS_PER_CORE * 128
NPAD = N_CORES * ROWS_PER_CORE            # 100352
MACROS = [8] * 12 + [2]                   # blocks per macro-tile (sum 98)
CHUNK_BLOCKS = 4                          # rows per MLP chunk = 512
MAGIC = 0x5F3759DF
MAGICF = float(MAGIC)

# segment groups: (n_channels, width, eq column offset, channel offset)
SEGS = [(64, 3, 0, 0), (32, 5, 192, 64), (16, 7, 352, 96)]

_EXPECTED_REP = np.concatenate(
    [np.repeat(np.arange(m) + off, 2 * l + 1)
     for l, (m, off) in enumerate([(128, 0), (64, 128), (32, 192), (16, 224)])]
)

# ------------------------------------------------------------- custom DVE op --
from concourse.dve_spec import Spec, Src0, Src1, C0, C1, C2, lower
from concourse.dve_uop import DveOpSpec
import concourse.dve_ops as dve_ops
from concourse.dve_ops import DveOp

# Newton rsqrt step: out = y*(C1 - C0*((v+C2)*y*y));  in0=v, in1=y
_nr_body = Src1 * (C1 - ((Src0 + C2) * (Src1 * Src1)) * C0)


def _nr_ref(in0, in1, s0, s1, imm2):
    y = in1.astype(np.float32)
    v = in0.astype(np.float32)
    return (y * (np.float32(s1) - ((v + np.float32(imm2)) * y * y) * np.float32(s0))
            ).astype(np.float32)


def _register_nr_op():
    name = "ANT_RSQRT_NR2"
    if name in dve_ops._SUB_OPCODE_FOR_NAME:
        for op in dve_ops.OPS:
            if op.name == name:
                return op
    spec = Spec(body=_nr_body, reference=_nr_ref)
    shas = {}
    row = 1 + len(dve_ops.OPS)
    for ver in ("v3", "v4"):
        s = DveOpSpec(name=name, opcode=row, uops=lower(spec, ver=ver), rd1_en=True)
        shas[ver] = s.sha(ver)
    op = DveOp(name, spec, subdim=False, uops_sha=shas)
    dve_ops.OPS.append(op)
    dve_ops._SUB_OPCODE_FOR_NAME[name] = row
    dve_ops.CUSTOM_DVE_SPECS[name] = spec
    return op


RSQRT_NR = _register_nr_op()


def _np_rsqrt_chain(v):
    """host model: seed + 2 NR (for reference/testing)."""
    t = v.view(np.int32).astype(np.float32)
    sb = (np.float32(-0.5) * t + np.float32(MAGICF)).astype(np.int32)
    y = _nr_ref(v, sb.view(np.float32), 0.5, 1.5, 0.0)
    y = _nr_ref(v, y, 0.5, 1.5, 0.0)
    return y


# ------------------------------------------------------------ kernel builder --
def _build_nc():
    nc = bacc.Bacc()

    x = nc.dram_tensor("x", [ROWS_PER_CORE, DIM], f32, kind="ExternalInput")
    out = nc.dram_tensor("out", [ROWS_PER_CORE, DIM], f32, kind="ExternalOutput")
    w1a_d = nc.dram_tensor("w1a", [128, 128], f32, kind="ExternalInput")
    w1b_d = nc.dram_tensor("w1b", [112, 128], f32, kind="ExternalInput")
    w2_d = nc.dram_tensor("w2p", [128, 128], f32, kind="ExternalInput")
    w3_d = nc.dram_tensor("w3p", [128, 128], f32, kind="ExternalInput")
    cmat_d = nc.dram_tensor("cmat", [128, 128], f32, kind="ExternalInput")
    onesd_d = nc.dram_tensor("onesd", [128, 128], f32, kind="ExternalInput")
    ident_d = nc.dram_tensor("ident", [128, 128], f32, kind="ExternalInput")
    b2_d = nc.dram_tensor("b2c", [128, 1], f32, kind="ExternalInput")
    b3_d = nc.dram_tensor("b3c", [128, 1], f32, kind="ExternalInput")

    # extra float consts used as activation bias (register like Bass.__init__)
    for _v in (MAGICF, float(EPS)):
        _t = nc.alloc_sbuf_tensor(f"const-f32-{_v}", [128, 1], f32)
        nc.gpsimd.memset(_t.ap(), _v)
        nc.const_aps.aps[(f32, _v)] = _t.ap()
    nc.all_engine_barrier()

    AF = mybir.ActivationFunctionType
    ALU = mybir.AluOpType
    AX = mybir.AxisListType

    from contextlib import ExitStack

    with tile.TileContext(nc) as tc:
        with ExitStack() as ctx:
            wpool = ctx.enter_context(tc.tile_pool(name="w", bufs=1))
            xpool = ctx.enter_context(tc.tile_pool(name="xp", bufs=2))
            opool = ctx.enter_context(tc.tile_pool(name="op", bufs=2))
            epool = ctx.enter_context(tc.tile_pool(name="ep", bufs=1))
            spool = ctx.enter_context(tc.tile_pool(name="sp", bufs=2))
            cpool = ctx.enter_context(tc.tile_pool(name="cp", bufs=2))
            ps_tp = ctx.enter_context(tc.tile_pool(name="ptp", bufs=3, space="PSUM"))
            ps_mm = ctx.enter_context(tc.tile_pool(name="pmm", bufs=2, space="PSUM"))
            ps_q = ctx.enter_context(tc.tile_pool(name="pq", bufs=2, space="PSUM"))

            w1a = wpool.tile([128, 128], f32, tag="w1a")
            nc.gpsimd.dma_start(out=w1a, in_=w1a_d[:, :])
            w1b = wpool.tile([112, 128], f32, tag="w1b")
            nc.gpsimd.dma_start(out=w1b, in_=w1b_d[:, :])
            w2p = wpool.tile([128, 128], f32, tag="w2p")
            nc.gpsimd.dma_start(out=w2p, in_=w2_d[:, :])
            w3p = wpool.tile([128, 128], f32, tag="w3p")
            nc.gpsimd.dma_start(out=w3p, in_=w3_d[:, :])
            cmat = wpool.tile([128, 128], f32, tag="cmat")
            nc.gpsimd.dma_start(out=cmat, in_=cmat_d[:, :])
            onesd = wpool.tile([128, 128], f32, tag="onesd")
            nc.gpsimd.dma_start(out=onesd, in_=onesd_d[:, :])
            ident = wpool.tile([128, 128], f32, tag="ident")
            nc.gpsimd.dma_start(out=ident, in_=ident_d[:, :])
            b2c = wpool.tile([128, 1], f32, tag="b2c")
            nc.gpsimd.dma_start(out=b2c, in_=b2_d[:, :])
            b3c = wpool.tile([128, 1], f32, tag="b3c")
            nc.gpsimd.dma_start(out=b3c, in_=b3_d[:, :])

            row0 = 0
            for nb in MACROS:
                R_rows = nb * 128
                xv = x[row0 : row0 + R_rows, :].rearrange("(p b) d -> p b d", b=nb)
                ov = out[row0 : row0 + R_rows, :].rearrange("(p b) d -> p b d", b=nb)
                row0 += R_rows

                X = xpool.tile([128, nb, DIM], f32, tag="X")
                nc.sync.dma_start(out=X, in_=xv)
                O = opool.tile([128, nb, DIM], f32, tag="O")

                # ---- eq path (rows on partitions) ----
                eq2 = epool.tile([128, nb, N_EQ], f32, tag="eq2")
                nc.scalar.activation(out=eq2, in_=X[:, :, N_INV:DIM], func=AF.Square)

                sumsq = spool.tile([128, nb, N_EQ_CH], f32, tag="sumsq")
                for (nch, w, eqoff, choff) in SEGS:
                    nc.vector.reduce_sum(
                        out=sumsq[:, :, choff : choff + nch],
                        in_=eq2[:, :, eqoff : eqoff + nch * w].rearrange(
                            "p b (c t) -> p b c t", t=w
                        ),
                        axis=AX.X,
                    )

                s1 = spool.tile([128, nb, N_EQ_CH], f32, tag="s1")
                nc.vector.tensor_scalar_add(out=s1, in0=sumsq, scalar1=1.0)

                # rsqrt chain: r = rsqrt(s1)
                seedb = spool.tile([128, nb, N_EQ_CH], i32, tag="seedb")
                nc.scalar.activation(
                    out=seedb, in_=s1.bitcast(i32), func=AF.Identity,
                    scale=-0.5, bias=MAGICF,
                )
                flat3 = lambda ap: ap.rearrange("p a b -> p (a b)")
                ry = spool.tile([128, nb, N_EQ_CH], f32, tag="ry")
                nc.vector._custom_dve(
                    RSQRT_NR, out=flat3(ry), in0=flat3(s1),
                    in1=flat3(seedb.bitcast(f32)), s0=0.5, s1=1.5, imm2=0.0,
                )
                r = spool.tile([128, nb, N_EQ_CH], f32, tag="r")
                nc.vector._custom_dve(
                    RSQRT_NR, out=flat3(r), in0=flat3(s1), in1=flat3(ry),
                    s0=0.5, s1=1.5, imm2=0.0,
                )

                # x11 = s1 * r - 1  (= sqrt(s1) - 1)
                x11 = spool.tile([128, nb, N_EQ_CH], f32, tag="x11")
                nc.vector.tensor_mul(x11, s1, r)
                nc.vector.tensor_scalar_add(out=x11, in0=x11, scalar1=-1.0)

                # x2 = eq * r[seg]  -> O[:, :, 128:]
                for (nch, w, eqoff, choff) in SEGS:
                    rbc = (
                        r[:, :, choff : choff + nch]
                        .unsqueeze(-1)
                        .broadcast_to((128, nb, nch, w))
                    )
                    nc.vector.tensor_mul(
                        O[:, :, N_INV + eqoff : N_INV + eqoff + nch * w].rearrange(
                            "p b (c t) -> p b c t", t=w
                        ),
                        X[:, :, N_INV + eqoff : N_INV + eqoff + nch * w].rearrange(
                            "p b (c t) -> p b c t", t=w
                        ),
                        rbc,
                    )

                # ---- MLP path (feature-major per chunk) ----
                for cb0 in range(0, nb, CHUNK_BLOCKS):
                    cnb = min(CHUNK_BLOCKS, nb - cb0)
                    R = cnb * 128

                    TPa = ps_tp.tile([128, R], f32, tag="tp")
                    TPb = ps_tp.tile([128, R], f32, tag="tp")
                    for j in range(cnb):
                        nc.tensor.transpose(
                            TPa[:, j * 128 : (j + 1) * 128], X[:, cb0 + j, 0:N_INV],
                            ident,
                        )
                        nc.tensor.transpose(
                            TPb[0:N_EQ_CH, j * 128 : (j + 1) * 128],
                            x11[:, cb0 + j, :], ident,
                        )
                    x1Ta = cpool.tile([128, R], f32, tag="x1Ta")
                    nc.vector.tensor_copy(x1Ta, TPa)
                    x1Tb = cpool.tile([112, R], f32, tag="x1Tb")
                    nc.vector.tensor_copy(x1Tb, TPb[0:N_EQ_CH, :])

                    H1 = ps_mm.tile([128, R], f32, tag="mm")
                    nc.tensor.matmul(H1, w1a, x1Ta, start=True, stop=False)
                    nc.tensor.matmul(H1, w1b, x1Tb, start=False, stop=True)

                    # LN1 (centering pre-folded into w1): rstd = rsqrt(mean(H1^2)+eps)
                    sq1 = cpool.tile([128, R], f32, tag="sq1")
                    nc.scalar.activation(out=sq1, in_=H1, func=AF.Square)
                    Q1 = ps_q.tile([128, R], f32, tag="qq")
                    nc.tensor.matmul(Q1, onesd, sq1, start=True, stop=True)
                    vg1 = cpool.tile([128, R], f32, tag="vg1")
                    nc.scalar.activation(out=vg1, in_=Q1, func=AF.Relu,
                                         bias=float(EPS))
                    sd1 = cpool.tile([128, R], i32, tag="sd1")
                    nc.scalar.activation(out=sd1, in_=vg1.bitcast(i32),
                                         func=AF.Identity, scale=-0.5, bias=MAGICF)
                    ry1 = cpool.tile([128, R], f32, tag="ry1")
                    nc.vector._custom_dve(
                        RSQRT_NR, out=ry1, in0=vg1, in1=sd1.bitcast(f32),
                        s0=0.5, s1=1.5, imm2=0.0,
                    )
                    rstd1 = cpool.tile([128, R], f32, tag="rstd1")
                    nc.vector._custom_dve(
                        RSQRT_NR, out=rstd1, in0=vg1, in1=ry1,
                        s0=0.5, s1=1.5, imm2=0.0,
                    )
                    hn1 = cpool.tile([128, R], f32, tag="hn1")
                    nc.vector.tensor_mul(hn1, H1, rstd1)

                    H2 = ps_mm.tile([128, R], f32, tag="mm")
                    nc.tensor.matmul(H2, w2p, hn1, start=True, stop=True)
                    av = cpool.tile([128, R], f32, tag="av")
                    nc.scalar.activation(out=av, in_=H2, func=AF.Silu, bias=b2c)

                    # LN2: explicit centering via C matmul
                    AC = ps_mm.tile([128, R], f32, tag="mm")
                    nc.tensor.matmul(AC, cmat, av, start=True, stop=True)
                    sq2 = cpool.tile([128, R], f32, tag="sq2")
                    nc.scalar.activation(out=sq2, in_=AC, func=AF.Square)
                    Q2 = ps_q.tile([128, R], f32, tag="qq")
                    nc.tensor.matmul(Q2, onesd, sq2, start=True, stop=True)
                    vg2 = cpool.tile([128, R], f32, tag="vg2")
                    nc.scalar.activation(out=vg2, in_=Q2, func=AF.Relu,
                                         bias=float(EPS))
                    sd2 = cpool.tile([128, R], i32, tag="sd2")
                    nc.scalar.activation(out=sd2, in_=vg2.bitcast(i32),
                                         func=AF.Identity, scale=-0.5, bias=MAGICF)
                    ry2 = cpool.tile([128, R], f32, tag="ry2")
                    nc.vector._custom_dve(
                        RSQRT_NR, out=ry2, in0=vg2, in1=sd2.bitcast(f32),
                        s0=0.5, s1=1.5, imm2=0.0,
                    )
                    rstd2 = cpool.tile([128, R], f32, tag="rstd2")
                    nc.vector._custom_dve(
                        RSQRT_NR, out=rstd2, in0=vg2, in1=ry2,
                        s0=0.5, s1=1.5, imm2=0.0,
                    )
                    hn2 = cpool.tile([128, R], f32, tag="hn2")
                    nc.vector.tensor_mul(hn2, AC, rstd2)

                    H3 = ps_mm.tile([128, R], f32, tag="mm")
                    nc.tensor.matmul(H3, w3p, hn2, start=True, stop=True)
                    h3b = cpool.tile([128, R], f32, tag="h3b")
                    nc.scalar.activation(out=h3b, in_=H3, func=AF.Identity,
                                         bias=b3c)

                    TPo = ps_tp.tile([128, R], f32, tag="tp")
                    for j in range(cnb):
                        nc.tensor.transpose(
                            TPo[:, j * 128 : (j + 1) * 128],
                            h3b[:, j * 128 : (j + 1) * 128], ident,
                        )
                    nc.vector.tensor_copy(
                        O[:, cb0 : cb0 + cnb, 0:N_INV],
                        TPo.rearrange("p (b j) -> p b j", j=128),
                    )

                nc.sync.dma_start(out=ov, in_=O)

    nc.finalize()
    return nc


_NC_CACHE = {}


def _get_nc():
    if "nc" not in _NC_CACHE:
        _NC_CACHE["nc"] = _build_nc()
    return _NC_CACHE["nc"]


# --------------------------------------------------------------- host driver --
def _prep_weights(w1, g1, beta1, w2, b2, g2, beta2, w3, b3):
    C = np.eye(128, dtype=np.float64) - 1.0 / 128.0
    w1p = w1.astype(np.float64) @ C                       # [240,128]
    w2p = (g1.astype(np.float64)[:, None] * w2.astype(np.float64))
    b2c = beta1.astype(np.float64) @ w2.astype(np.float64) + b2.astype(np.float64)
    w3p = (g2.astype(np.float64)[:, None] * w3.astype(np.float64))
    b3c = beta2.astype(np.float64) @ w3.astype(np.float64) + b3.astype(np.float64)
    return {
        "w1a": np.ascontiguousarray(w1p[0:128], dtype=np.float32),
        "w1b": np.ascontiguousarray(w1p[128:240], dtype=np.float32),
        "w2p": w2p.astype(np.float32),
        "w3p": w3p.astype(np.float32),
        "cmat": C.astype(np.float32),
        "onesd": np.full((128, 128), 1.0 / 128.0, dtype=np.float32),
        "ident": np.eye(128, dtype=np.float32),
        "b2c": b2c.astype(np.float32).reshape(128, 1),
        "b3c": b3c.astype(np.float32).reshape(128, 1),
    }


def _np_reference(ten, w1, g1, beta1, w2, b2, g2, beta2, w3, b3):
    """Pure-numpy fallback (used only if rep_layout is unexpected)."""
    x10 = ten[:, :N_INV]
    eq = ten[:, N_INV:]
    sumsq = np.zeros((ten.shape[0], N_EQ_CH), np.float32)
    for (nch, w, eqoff, choff) in SEGS:
        sumsq[:, choff:choff + nch] = (
            (eq[:, eqoff:eqoff + nch * w].reshape(-1, nch, w) ** 2).sum(-1)
        )
    d = np.sqrt(sumsq + 1.0)
    x11 = d - 1.0
    x1 = np.concatenate([x10, x11], 1)
    seg = np.concatenate([np.repeat(np.arange(nch) + choff, w)
                          for (nch, w, eqoff, choff) in SEGS])
    x2 = eq / d[:, seg]

    def ln(h, g, b):
        mu = h.mean(-1, keepdims=True)
        var = h.var(-1, keepdims=True)
        return (h - mu) / np.sqrt(var + EPS) * g + b

    h = x1 @ w1
    h = ln(h, g1, beta1)
    h = h @ w2 + b2
    h = h / (1 + np.exp(-h)) * 1.0 if False else h * (1 / (1 + np.exp(-h)))
    h = ln(h, g2, beta2)
    h = h @ w3 + b3
    return np.concatenate([h, x2], 1).astype(np.float32)


def kernel(ten, rep_layout, w1, g1, beta1, w2, b2, g2, beta2, w3, b3):
    ten = np.asarray(ten, dtype=np.float32)
    args = [np.asarray(a) for a in (w1, g1, beta1, w2, b2, g2, beta2, w3, b3)]
    w1, g1, beta1, w2, b2, g2, beta2, w3, b3 = [a.astype(np.float32) for a in args]

    if not np.array_equal(np.asarray(rep_layout).astype(np.int64), _EXPECTED_REP):
        return _np_reference(ten, w1, g1, beta1, w2, b2, g2, beta2, w3, b3)

    wmap = _prep_weights(w1, g1, beta1, w2, b2, g2, beta2, w3, b3)

    xpad = np.zeros((NPAD, DIM), dtype=np.float32)
    xpad[:N] = ten
    shards = xpad.reshape(N_CORES, ROWS_PER_CORE, DIM)

    nc = _get_nc()
    in_maps = [dict(wmap, x=np.ascontiguousarray(shards[c]))
               for c in range(N_CORES)]
    res = run_bass_kernel_spmd(nc, in_maps, list(range(N_CORES))).results
    outp = np.concatenate([res[c]["out"] for c in range(N_CORES)], axis=0)
    return np.ascontiguousarray(outp[:N])


# revision 13
# speedup vs baseline: 9.7028x; 9.7028x over previous
"""Trainium2 Bass kernel for nn_EvMLP (segment_reduce EvNorm + invariant MLP).

Self-contained: hardcodes shapes/sharding. Accepts FULL inputs, returns FULL
output; shards the node dim N across 8 NeuronCores (pure data parallel).

Math (per row of ten [N, 592]):
  x10 = ten[:128]; eq = ten[128:]
  sumsq[c] = sum of eq^2 over segment c   (64x3, 32x5, 16x7 runs)
  d = sqrt(sumsq+1);  x11 = d-1;  x2 = eq / d[seg]
  x1 = [x10, x11]  (240)
  h = LN(x1@w1, g1, b1); h = silu(h@w2+b2); h = LN(h, g2, b2n); h = h@w3+b3
  out = [h, x2]

Key implementation ideas:
  - rows-on-partitions for the eq path (squares / segment reduce / divide)
  - feature-major MLP via PE transposes; LayerNorm mean-centering folded into
    the weights host-side (w @ C); LN variance via ones/128 matmul broadcast
  - rsqrt without ACT table switching: quake-style seed computed with ACT
    Identity dtype-conversion trick + 2 custom fused DVE Newton steps
  - only Silu/Square/Identity/Relu activation funcs -> single ACT table set
"""
import sys

sys.path.insert(0, "/opt/trn_rl_repo")

import numpy as np

import concourse.bass as bass
import concourse.bacc as bacc
import concourse.tile as tile
from concourse import mybir
from concourse.bass_utils import run_bass_kernel_spmd

f32 = mybir.dt.float32
i32 = mybir.dt.int32

# ---------------------------------------------------------------- constants --
N = 100000
DIM = 592
N_INV = 128
N_EQ_CH = 112
N_EQ = 464
EPS = 1e-5
N_CORES = 8
BLOCKS_PER_CORE = 98                      # 98*128 = 12544 rows/core
ROWS_PER_CORE = BLOCKS_PER_CORE * 128
NPAD = N_CORES * ROWS_PER_CORE            # 100352
MACROS = [8] * 12 + [2]                   # blocks per macro-tile (sum 98)
CHUNK_BLOCKS = 4                          # rows per MLP chunk = 512
MAGIC = 0x5F3759DF
MAGICF = float(MAGIC)

# segment groups: (n_channels, width, eq column offset, channel offset)
SEGS = [(64, 3, 0, 0), (32, 5, 192, 64), (16, 7, 352, 96)]

_EXPECTED_REP = np.concatenate(
    [np.repeat(np.arange(m) + off, 2 * l + 1)
     for l, (m, off) in enumerate([(128, 0), (64, 128), (32, 192), (16, 224)])]
)

# ------------------------------------------------------------- custom DVE op --
from concourse.dve_spec import Spec, Src0, Src1, C0, C1, C2, lower
from concourse.dve_uop import DveOpSpec
import concourse.dve_ops as dve_ops
from concourse.dve_ops import DveOp

# Newton rsqrt step: out = y*(C1 - C0*((v+C2)*y*y));  in0=v, in1=y
_nr_body = Src1 * (C1 - ((Src0 + C2) * (Src1 * Src1)) * C0)


def _nr_ref(in0, in1, s0, s1, imm2):
    y = in1.astype(np.float32)
    v = in0.astype(np.float32)
    return (y * (np.float32(s1) - ((v + np.float32(imm2)) * y * y) * np.float32(s0))
            ).astype(np.float32)


def _register_nr_op():
    name = "ANT_RSQRT_NR2"
    if name in dve_ops._SUB_OPCODE_FOR_NAME:
        for op in dve_ops.OPS:
            if op.name == name:
                return op
    spec = Spec(body=_nr_body, reference=_nr_ref)
    shas = {}
    row = 1 + len(dve_ops.OPS)
    for ver in ("v3", "v4"):
        s = DveOpSpec(name=name, opcode=row, uops=lower(spec, ver=ver), rd1_en=True)
        shas[ver] = s.sha(ver)
    op = DveOp(name, spec, subdim=False, uops_sha=shas)
    dve_ops.OPS.append(op)
    dve_ops._SUB_OPCODE_FOR_NAME[name] = row
    dve_ops.CUSTOM_DVE_SPECS[name] = spec
    return op


RSQRT_NR = _register_nr_op()


def _register_mulsub1():
    name = "ANT_MUL_SUB1"
    if name in dve_ops._SUB_OPCODE_FOR_NAME:
        for op in dve_ops.OPS:
            if op.name == name:
                return op
    from concourse.dve_spec import One
    spec = Spec(
        body=(Src0 * Src1) - One,
        reference=lambda in0, in1, s0, s1, imm2: (
            in0.astype(np.float32) * in1 - np.float32(1.0)
        ).astype(np.float32),
    )
    shas = {}
    row = 1 + len(dve_ops.OPS)
    for ver in ("v3", "v4"):
        sp = DveOpSpec(name=name, opcode=row, uops=lower(spec, ver=ver), rd1_en=True)
        shas[ver] = sp.sha(ver)
    op = DveOp(name, spec, subdim=False, uops_sha=shas)
    dve_ops.OPS.append(op)
    dve_ops._SUB_OPCODE_FOR_NAME[name] = row
    dve_ops.CUSTOM_DVE_SPECS[name] = spec
    return op


MUL_SUB1 = _register_mulsub1()


def _np_rsqrt_chain(v):
    """host model: seed + 2 NR (for reference/testing)."""
    t = v.view(np.int32).astype(np.float32)
    sb = (np.float32(-0.5) * t + np.float32(MAGICF)).astype(np.int32)
    y = _nr_ref(v, sb.view(np.float32), 0.5, 1.5, 0.0)
    y = _nr_ref(v, y, 0.5, 1.5, 0.0)
    return y


# ------------------------------------------------------------ kernel builder --
def _build_nc():
    nc = bacc.Bacc()

    x = nc.dram_tensor("x", [ROWS_PER_CORE, DIM], f32, kind="ExternalInput")
    out = nc.dram_tensor("out", [ROWS_PER_CORE, DIM], f32, kind="ExternalOutput")
    w1a_d = nc.dram_tensor("w1a", [128, 128], f32, kind="ExternalInput")
    w1b_d = nc.dram_tensor("w1b", [112, 128], f32, kind="ExternalInput")
    w2_d = nc.dram_tensor("w2p", [128, 128], f32, kind="ExternalInput")
    w3_d = nc.dram_tensor("w3p", [128, 128], f32, kind="ExternalInput")
    cmat_d = nc.dram_tensor("cmat", [128, 128], f32, kind="ExternalInput")
    onesd_d = nc.dram_tensor("onesd", [128, 128], f32, kind="ExternalInput")
    ident_d = nc.dram_tensor("ident", [128, 128], f32, kind="ExternalInput")
    b2_d = nc.dram_tensor("b2c", [128, 1], f32, kind="ExternalInput")
    b3_d = nc.dram_tensor("b3c", [128, 1], f32, kind="ExternalInput")
    b3nat_d = nc.dram_tensor("b3nat", [128, 512], f32, kind="ExternalInput")

    # extra float consts used as activation bias (register like Bass.__init__)
    for _v in (MAGICF, float(EPS)):
        _t = nc.alloc_sbuf_tensor(f"const-f32-{_v}", [128, 1], f32)
        nc.gpsimd.memset(_t.ap(), _v)
        nc.const_aps.aps[(f32, _v)] = _t.ap()
    nc.all_engine_barrier()

    AF = mybir.ActivationFunctionType
    ALU = mybir.AluOpType
    AX = mybir.AxisListType

    from contextlib import ExitStack

    with tile.TileContext(nc) as tc:
        with ExitStack() as ctx:
            wpool = ctx.enter_context(tc.tile_pool(name="w", bufs=1))
            xpool = ctx.enter_context(tc.tile_pool(name="xp", bufs=2))
            opool = ctx.enter_context(tc.tile_pool(name="op", bufs=2))
            epool = ctx.enter_context(tc.tile_pool(name="ep", bufs=1))
            spool = ctx.enter_context(tc.tile_pool(name="sp", bufs=2))
            cpool = ctx.enter_context(tc.tile_pool(name="cp", bufs=3))
            ps_tp = ctx.enter_context(tc.tile_pool(name="ptp", bufs=3, space="PSUM"))
            ps_mm = ctx.enter_context(tc.tile_pool(name="pmm", bufs=3, space="PSUM"))
            ps_q = ctx.enter_context(tc.tile_pool(name="pq", bufs=2, space="PSUM"))

            w1a = wpool.tile([128, 128], f32, tag="w1a")
            nc.gpsimd.dma_start(out=w1a, in_=w1a_d[:, :])
            w1b = wpool.tile([112, 128], f32, tag="w1b")
            nc.gpsimd.dma_start(out=w1b, in_=w1b_d[:, :])
            w2p = wpool.tile([128, 128], f32, tag="w2p")
            nc.gpsimd.dma_start(out=w2p, in_=w2_d[:, :])
            w3p = wpool.tile([128, 128], f32, tag="w3p")
            nc.gpsimd.dma_start(out=w3p, in_=w3_d[:, :])
            cmat = wpool.tile([128, 128], f32, tag="cmat")
            nc.gpsimd.dma_start(out=cmat, in_=cmat_d[:, :])
            onesd = wpool.tile([128, 128], f32, tag="onesd")
            nc.gpsimd.dma_start(out=onesd, in_=onesd_d[:, :])
            ident = wpool.tile([128, 128], f32, tag="ident")
            nc.gpsimd.dma_start(out=ident, in_=ident_d[:, :])
            b2c = wpool.tile([128, 1], f32, tag="b2c")
            nc.gpsimd.dma_start(out=b2c, in_=b2_d[:, :])
            b3c = wpool.tile([128, 1], f32, tag="b3c")
            nc.gpsimd.dma_start(out=b3c, in_=b3_d[:, :])
            b3nat = wpool.tile([128, 512], f32, tag="b3nat")
            nc.gpsimd.dma_start(out=b3nat, in_=b3nat_d[:, :])

            row0 = 0
            for nb in MACROS:
                R_rows = nb * 128
                xv = x[row0 : row0 + R_rows, :].rearrange("(p b) d -> p b d", b=nb)
                ov = out[row0 : row0 + R_rows, :].rearrange("(p b) d -> p b d", b=nb)
                row0 += R_rows

                X = xpool.tile([128, nb, DIM], f32, tag="X")
                nc.sync.dma_start(out=X, in_=xv)
                O = opool.tile([128, nb, DIM], f32, tag="O")

                # ---- eq path (rows on partitions) ----
                eq2 = epool.tile([128, nb, N_EQ], f32, tag="eq2")
                nbh = (nb + 1) // 2
                nc.scalar.activation(
                    out=eq2[:, 0:nbh, :], in_=X[:, 0:nbh, N_INV:DIM], func=AF.Square
                )
                nc.gpsimd.tensor_tensor(
                    out=eq2[:, nbh:nb, :], in0=X[:, nbh:nb, N_INV:DIM],
                    in1=X[:, nbh:nb, N_INV:DIM], op=mybir.AluOpType.mult,
                )

                sumsq = spool.tile([128, nb, N_EQ_CH], f32, tag="sumsq")
                for (nch, w, eqoff, choff) in SEGS:
                    nc.vector.reduce_sum(
                        out=sumsq[:, :, choff : choff + nch],
                        in_=eq2[:, :, eqoff : eqoff + nch * w].rearrange(
                            "p b (c t) -> p b c t", t=w
                        ),
                        axis=AX.X,
                    )

                s1 = spool.tile([128, nb, N_EQ_CH], f32, tag="s1")
                nc.vector.tensor_scalar_add(out=s1, in0=sumsq, scalar1=1.0)

                # rsqrt chain: r = rsqrt(s1)
                seedb = spool.tile([128, nb, N_EQ_CH], i32, tag="seedb")
                nc.scalar.activation(
                    out=seedb, in_=s1.bitcast(i32), func=AF.Identity,
                    scale=-0.5, bias=MAGICF,
                )
                flat3 = lambda ap: ap.rearrange("p a b -> p (a b)")
                ry = spool.tile([128, nb, N_EQ_CH], f32, tag="ry")
                nc.vector._custom_dve(
                    RSQRT_NR, out=flat3(ry), in0=flat3(s1),
                    in1=flat3(seedb.bitcast(f32)), s0=0.5, s1=1.5, imm2=0.0,
                )
                r = spool.tile([128, nb, N_EQ_CH], f32, tag="r")
                nc.vector._custom_dve(
                    RSQRT_NR, out=flat3(r), in0=flat3(s1), in1=flat3(ry),
                    s0=0.5, s1=1.5, imm2=0.0,
                )

                # x11 = s1 * r - 1  (= sqrt(s1) - 1)
                x11 = spool.tile([128, nb, N_EQ_CH], f32, tag="x11")
                nc.vector._custom_dve(
                    MUL_SUB1, out=flat3(x11), in0=flat3(s1), in1=flat3(r),
                    s0=0.0, s1=0.0, imm2=0.0,
                )

                # x2 = eq * r[seg]  -> O[:, :, 128:]
                for (nch, w, eqoff, choff) in SEGS:
                    rbc = (
                        r[:, :, choff : choff + nch]
                        .unsqueeze(-1)
                        .broadcast_to((128, nb, nch, w))
                    )
                    nc.gpsimd.tensor_tensor(
                        out=O[:, :, N_INV + eqoff : N_INV + eqoff + nch * w].rearrange(
                            "p b (c t) -> p b c t", t=w
                        ),
                        in0=X[:, :, N_INV + eqoff : N_INV + eqoff + nch * w].rearrange(
                            "p b (c t) -> p b c t", t=w
                        ),
                        in1=rbc,
                        op=mybir.AluOpType.mult,
                    )

                # ---- MLP path (feature-major, stages interleaved across chunks) ----
                chunks = []
                for cb0 in range(0, nb, CHUNK_BLOCKS):
                    cnb = min(CHUNK_BLOCKS, nb - cb0)
                    chunks.append((cb0, cnb, cnb * 128))

                # stage T: all input transposes, then copies
                st = {}
                for ci, (cb0, cnb, R) in enumerate(chunks):
                    TPa = ps_tp.tile([128, R], f32, tag="tp")
                    TPb = ps_tp.tile([128, R], f32, tag="tp")
                    for j in range(cnb):
                        nc.tensor.transpose(
                            TPa[:, j * 128 : (j + 1) * 128], X[:, cb0 + j, 0:N_INV],
                            ident,
                        )
                    for j in range(cnb):
                        nc.tensor.transpose(
                            TPb[0:N_EQ_CH, j * 128 : (j + 1) * 128],
                            x11[:, cb0 + j, :], ident,
                        )
                    st[ci] = (TPa, TPb)
                xt = {}
                for ci, (cb0, cnb, R) in enumerate(chunks):
                    TPa, TPb = st[ci]
                    x1Ta = cpool.tile([128, R], f32, tag="x1Ta")
                    nc.scalar.activation(out=x1Ta, in_=TPa, func=AF.Identity)
                    x1Tb = cpool.tile([112, R], f32, tag="x1Tb")
                    nc.scalar.activation(out=x1Tb, in_=TPb[0:N_EQ_CH, :],
                                         func=AF.Identity)
                    xt[ci] = (x1Ta, x1Tb)

                # stage M1 + LN1
                h1 = {}
                for ci, (cb0, cnb, R) in enumerate(chunks):
                    x1Ta, x1Tb = xt[ci]
                    H1 = ps_mm.tile([128, R], f32, tag="mm")
                    nc.tensor.matmul(H1, w1a, x1Ta, start=True, stop=False)
                    nc.tensor.matmul(H1, w1b, x1Tb, start=False, stop=True)
                    h1[ci] = H1
                ln1 = {}
                for ci, (cb0, cnb, R) in enumerate(chunks):
                    H1 = h1[ci]
                    h1sb = cpool.tile([128, R], f32, tag="hn1")
                    nc.scalar.activation(out=h1sb, in_=H1, func=AF.Identity)
                    sq1 = cpool.tile([128, R], f32, tag="sq1")
                    nc.scalar.activation(out=sq1, in_=H1, func=AF.Square)
                    Q1 = ps_q.tile([128, R], f32, tag="qq")
                    nc.tensor.matmul(Q1, onesd, sq1, start=True, stop=True)
                    sd1 = cpool.tile([128, R], i32, tag="sd1")
                    nc.scalar.activation(out=sd1, in_=Q1.bitcast(i32),
                                         func=AF.Identity, scale=-0.5, bias=MAGICF)
                    rstd1 = cpool.tile([128, R], f32, tag="rstd1")
                    nc.vector._custom_dve(
                        RSQRT_NR, out=rstd1, in0=Q1, in1=sd1.bitcast(f32),
                        s0=0.5, s1=1.5, imm2=float(EPS),
                    )
                    nc.vector._custom_dve(
                        RSQRT_NR, out=sd1.bitcast(f32), in0=Q1, in1=rstd1,
                        s0=0.5, s1=1.5, imm2=float(EPS),
                    )
                    ln1[ci] = (h1sb, sd1)

                # stage M2 (on unscaled h1; rstd1 commutes through as column scale)
                ln2 = {}
                for ci, (cb0, cnb, R) in enumerate(chunks):
                    h1sb, sd1 = ln1[ci]
                    H2 = ps_mm.tile([128, R], f32, tag="mm")
                    nc.tensor.matmul(H2, w2p, h1sb, start=True, stop=True)
                    av = cpool.tile([128, R], f32, tag="av")
                    nc.vector.tensor_mul(av, H2, sd1.bitcast(f32))
                    nc.scalar.activation(out=av, in_=av, func=AF.Silu, bias=b2c)
                    AC = ps_mm.tile([128, R], f32, tag="mm")
                    nc.tensor.matmul(AC, cmat, av, start=True, stop=True)
                    sq2 = cpool.tile([128, R], f32, tag="sq2")
                    nc.scalar.activation(out=sq2, in_=AC, func=AF.Square)
                    Q2 = ps_q.tile([128, R], f32, tag="qq")
                    nc.tensor.matmul(Q2, onesd, sq2, start=True, stop=True)
                    sd2 = cpool.tile([128, R], i32, tag="sd2")
                    nc.scalar.activation(out=sd2, in_=Q2.bitcast(i32),
                                         func=AF.Identity, scale=-0.5, bias=MAGICF)
                    rstd2 = cpool.tile([128, R], f32, tag="rstd2")
                    nc.vector._custom_dve(
                        RSQRT_NR, out=rstd2, in0=Q2, in1=sd2.bitcast(f32),
                        s0=0.5, s1=1.5, imm2=float(EPS),
                    )
                    nc.vector._custom_dve(
                        RSQRT_NR, out=sd2.bitcast(f32), in0=Q2, in1=rstd2,
                        s0=0.5, s1=1.5, imm2=float(EPS),
                    )
                    hn2 = cpool.tile([128, R], f32, tag="hn2")
                    nc.vector.tensor_mul(hn2, AC, sd2.bitcast(f32))
                    ln2[ci] = hn2

                # stage M3: natural-orientation output via lhsT=hn2 blocks
                for ci, (cb0, cnb, R) in enumerate(chunks):
                    hn2 = ln2[ci]
                    H3n = ps_mm.tile([128, R], f32, tag="mm")
                    for j in range(cnb):
                        nc.tensor.matmul(
                            H3n[:, j * 128 : (j + 1) * 128],
                            hn2[:, j * 128 : (j + 1) * 128], w3p,
                            start=True, stop=True,
                        )
                    nc.vector.tensor_add(
                        O[:, cb0 : cb0 + cnb, 0:N_INV],
                        H3n.rearrange("p (b j) -> p b j", j=128),
                        b3nat[:, 0:R].rearrange("p (b j) -> p b j", j=128),
                    )

                nc.sync.dma_start(out=ov, in_=O)

    nc.finalize()
    return nc


_NC_CACHE = {}


def _get_nc():
    if "nc" not in _NC_CACHE:
        _NC_CACHE["nc"] = _build_nc()
    return _NC_CACHE["nc"]


# --------------------------------------------------------------- host driver --
def _prep_weights(w1, g1, beta1, w2, b2, g2, beta2, w3, b3):
    C = np.eye(128, dtype=np.float64) - 1.0 / 128.0
    w1p = w1.astype(np.float64) @ C                       # [240,128]
    w2p = (g1.astype(np.float64)[:, None] * w2.astype(np.float64))
    b2c = beta1.astype(np.float64) @ w2.astype(np.float64) + b2.astype(np.float64)
    w3p = (g2.astype(np.float64)[:, None] * w3.astype(np.float64))
    b3c = beta2.astype(np.float64) @ w3.astype(np.float64) + b3.astype(np.float64)
    return {
        "w1a": np.ascontiguousarray(w1p[0:128], dtype=np.float32),
        "w1b": np.ascontiguousarray(w1p[128:240], dtype=np.float32),
        "w2p": w2p.astype(np.float32),
        "w3p": w3p.astype(np.float32),
        "cmat": C.astype(np.float32),
        "onesd": np.full((128, 128), 1.0 / 128.0, dtype=np.float32),
        "ident": np.eye(128, dtype=np.float32),
        "b2c": b2c.astype(np.float32).reshape(128, 1),
        "b3c": b3c.astype(np.float32).reshape(128, 1),
        "b3nat": np.tile(b3c.astype(np.float32), (128, 4)),
    }


def _np_reference(ten, w1, g1, beta1, w2, b2, g2, beta2, w3, b3):
    """Pure-numpy fallback (used only if rep_layout is unexpected)."""
    x10 = ten[:, :N_INV]
    eq = ten[:, N_INV:]
    sumsq = np.zeros((ten.shape[0], N_EQ_CH), np.float32)
    for (nch, w, eqoff, choff) in SEGS:
        sumsq[:, choff:choff + nch] = (
            (eq[:, eqoff:eqoff + nch * w].reshape(-1, nch, w) ** 2).sum(-1)
        )
    d = np.sqrt(sumsq + 1.0)
    x11 = d - 1.0
    x1 = np.concatenate([x10, x11], 1)
    seg = np.concatenate([np.repeat(np.arange(nch) + choff, w)
                          for (nch, w, eqoff, choff) in SEGS])
    x2 = eq / d[:, seg]

    def ln(h, g, b):
        mu = h.mean(-1, keepdims=True)
        var = h.var(-1, keepdims=True)
        return (h - mu) / np.sqrt(var + EPS) * g + b

    h = x1 @ w1
    h = ln(h, g1, beta1)
    h = h @ w2 + b2
    h = h / (1 + np.exp(-h)) * 1.0 if False else h * (1 / (1 + np.exp(-h)))
    h = ln(h, g2, beta2)
    h = h @ w3 + b3
    return np.concatenate([h, x2], 1).astype(np.float32)


def kernel(ten, rep_layout, w1, g1, beta1, w2, b2, g2, beta2, w3, b3):
    ten = np.asarray(ten, dtype=np.float32)
    args = [np.asarray(a) for a in (w1, g1, beta1, w2, b2, g2, beta2, w3, b3)]
    w1, g1, beta1, w2, b2, g2, beta2, w3, b3 = [a.astype(np.float32) for a in args]

    if not np.array_equal(np.asarray(rep_layout).astype(np.int64), _EXPECTED_REP):
        return _np_reference(ten, w1, g1, beta1, w2, b2, g2, beta2, w3, b3)

    wmap = _prep_weights(w1, g1, beta1, w2, b2, g2, beta2, w3, b3)

    xpad = np.zeros((NPAD, DIM), dtype=np.float32)
    xpad[:N] = ten
    shards = xpad.reshape(N_CORES, ROWS_PER_CORE, DIM)

    nc = _get_nc()
    in_maps = [dict(wmap, x=np.ascontiguousarray(shards[c]))
               for c in range(N_CORES)]
    res = run_bass_kernel_spmd(nc, in_maps, list(range(N_CORES))).results
    outp = np.concatenate([res[c]["out"] for c in range(N_CORES)], axis=0)
    return np.ascontiguousarray(outp[:N])
